# revision 21
# baseline (speedup 1.0000x reference)
"""DGCNN edge-conv graph-feature module on Trainium2 (Bass/Tile), v2.

Problem: for each batch (B=8): F-space KNN (k=20) over N=4096 points (C=64),
gather neighbor features, edge-MLP (128->128->64->64 with relu), max-pool
over the 20 neighbors -> (4096, 64).

Sharding: data-parallel over batch across the 8 NeuronCores (one batch each,
SPMD single NEFF).

v2 changes vs v1 (1.57 ms):
  - distance matmuls in fp16 (1 cyc/row vs 4 for fp32).
  - index compaction via quantized sort keys: key = q*4096 + j packed in
    exact-int fp32 (q = clip(round(64 d + 3072), 0, 4095)); the top-20-of-64
    merge runs on keys, so indices fall out of a cheap `mod 4096` — no
    GPSIMD local_scatter, no cumsum chain.
  - neighbor gather via gpsimd.ap_gather from a duplicated-fp16-in-u32
    SBUF table (Q7 cores move data; no per-pair DMA descriptor generation,
    which was ~800 us of SWDGE ucode in v1).
  - pair order is (k, q128) so the v-broadcast matmul uses one fixed
    0-stride identity AP and the K max-pool reduces over contiguous halves.
  - engine rebalance: PSUM evacuations on ACT (relu fused), max-pool tree +
    key build on GPSIMD, top-k scans + merge on DVE.
"""

import os
import sys

for _p in ("/opt/trn_rl_repo", "/root/.axon_site/_ro/trn_rl_repo"):
    if os.path.isdir(_p) and _p not in sys.path:
        sys.path.insert(0, _p)

import numpy as np

import concourse.bass as bass
import concourse.mybir as mybir
from concourse import bacc
from concourse.bass_utils import run_bass_kernel_spmd
from concourse.masks import make_identity
from concourse.tile import TileContext

f32 = mybir.dt.float32
f16 = mybir.dt.float16
i16 = mybir.dt.int16
u16 = mybir.dt.uint16
u32 = mybir.dt.uint32

B, N, C, K = 8, 4096, 64, 20
C1, C2, C3 = 128, 64, 64
NT = N // 128              # point tiles per core
NBLK = N // 512            # candidate blocks per tile
NCAND = NBLK * 8           # merge candidates per row
PAIRS = 128 * K            # pairs per point tile (2560)
GROUP = 4                  # tiles per key/fold/gather/mlp phase group
NEG = -1e30
QS = 64.0                  # key quantization scale
QOFF = 3072.0              # key quantization offset

AluOp = mybir.AluOpType
Act = mybir.ActivationFunctionType


def build_nc(nt=NT, stage=None):
    if stage is None:
        stage = int(os.environ.get("KM_STAGE", "9"))
    nc = bacc.Bacc(None, target_bir_lowering=False)

    pts = nc.declare_dram_parameter("points", [N, C], f32, isOutput=False)
    w1 = nc.declare_dram_parameter("W1", [C1, 2 * C], f32, isOutput=False)
    b1 = nc.declare_dram_parameter("b1", [C1], f32, isOutput=False)
    w2 = nc.declare_dram_parameter("W2", [C2, C1], f32, isOutput=False)
    b2 = nc.declare_dram_parameter("b2", [C2], f32, isOutput=False)
    w3 = nc.declare_dram_parameter("W3", [C3, C2], f32, isOutput=False)
    b3 = nc.declare_dram_parameter("b3", [C3], f32, isOutput=False)
    out = nc.declare_dram_parameter("out", [N, C3], f32, isOutput=True)

    pool_chain = []  # GPSIMD extended-ISA ops, chained to batch ucode libraries

    with TileContext(nc) as tc:
        with tc.tile_pool(name="const", bufs=1) as cp:
            ident = cp.tile([128, 128], f32)
            make_identity(nc, ident)
            ident16 = cp.tile([128, 128], f16)
            nc.vector.tensor_copy(ident16, ident)

            # ---- persistent tensors
            xq16 = cp.tile([C + 1, N], f16)   # rows 0:64 = xT, row 64 = ones
            xc16 = cp.tile([C + 1, N], f16)   # rows 0:64 = 2*xT, row 64 = -|x|^2
            x_dup = cp.tile([128, N], u32)    # rows 0:64: fp16 x duplicated
            v_all = cp.tile([128, nt, C1], f16)
            w1t16 = cp.tile([2 * C, C1], f16)
            w2t16 = cp.tile([C1, C2], f16)
            w3t16 = cp.tile([C2, C3], f16)
            b2_col = cp.tile([C2, 1], f32)
            b3_col = cp.tile([C3, 1], f32)
            offs = cp.tile([128, NCAND], u16)  # slot -> 512*block
            ilist = cp.tile([128, nt, K], i16)
            wrap = cp.tile([128, nt, 160], i16)

            nc.sync.dma_start(out=b2_col, in_=b2.ap().rearrange("(c a) -> c a", a=1))
            nc.sync.dma_start(out=b3_col, in_=b3.ap().rearrange("(c a) -> c a", a=1))
            nc.gpsimd.iota(offs, pattern=[[512, NBLK], [0, 8]], base=0,
                           channel_multiplier=0)
            nc.vector.memset(xq16[C:C + 1, :], 1.0)
            nc.vector.memset(x_dup, 0.0)

            # ---- setup (freed after): load, transpose, weights, v
            with tc.tile_pool(name="setup", bufs=1) as sp, \
                 tc.tile_pool(name="setup_ps", bufs=2, space="PSUM") as sps:
                x_sb = sp.tile([128, nt, C], f32)
                src = bass.AP(
                    tensor=pts.ap().tensor, offset=0,
                    ap=[[C, 128], [128 * C, nt], [1, C]],
                )
                nc.sync.dma_start(out=x_sb, in_=src)

                w1_sb = sp.tile([C1, 2 * C], f32)
                nc.sync.dma_start(out=w1_sb, in_=w1[:, :])
                w2_sb = sp.tile([C2, C1], f32)
                nc.sync.dma_start(out=w2_sb, in_=w2[:, :])
                w3_sb = sp.tile([C3, C2], f32)
                nc.sync.dma_start(out=w3_sb, in_=w3[:, :])

                xTaug = sp.tile([C + 1, N], f32)
                nc.vector.memset(xTaug[C:C + 1, :], 1.0)

                # transpose W1, W2, W3 (fp16 copies)
                p = sps.tile([128, 128], f32, tag="tp")
                nc.tensor.transpose(p, w1_sb, ident)
                nc.vector.tensor_copy(w1t16, p)

                p = sps.tile([128, 128], f32, tag="tp")
                nc.tensor.transpose(p[:, 0:C2], w2_sb, ident[0:C2, 0:C2])
                nc.vector.tensor_copy(w2t16, p[0:C1, 0:C2])

                p = sps.tile([128, 128], f32, tag="tp")
                nc.tensor.transpose(p[0:C2, 0:C3], w3_sb[:, :], ident[0:C2, 0:C2])
                nc.vector.tensor_copy(w3t16, p[0:C2, 0:C3])

                # Wv_aug [65, C1]: rows 0:64 = (W1c - W1e)^T, row 64 = b1
                wv = sp.tile([C + 1, C1], f32)
                delta = sp.tile([C1, C], f32)
                nc.vector.tensor_tensor(
                    out=delta, in0=w1_sb[:, C:2 * C], in1=w1_sb[:, 0:C],
                    op=AluOp.subtract)
                p = sps.tile([128, 128], f32, tag="tp")
                nc.tensor.transpose(p[0:C, :], delta, ident)
                nc.vector.tensor_copy(wv[0:C, :], p[0:C, :])
                nc.sync.dma_start(out=wv[C:C + 1, :],
                                  in_=b1.ap().rearrange("(a c) -> a c", a=1))

                # per point-tile: transpose x -> xTaug, xq16, xc16
                for t in range(nt):
                    p = sps.tile([128, 128], f32, tag="tp")
                    nc.tensor.transpose(p[0:C, :], x_sb[:, t, :], ident)
                    sl = slice(t * 128, (t + 1) * 128)
                    nc.vector.tensor_copy(xTaug[0:C, sl], p[0:C, :])
                    nc.vector.tensor_copy(xq16[0:C, sl], p[0:C, :])
                    nc.vector.tensor_scalar_mul(xc16[0:C, sl], p[0:C, :], 2.0)

                # -|x_j|^2 row (fp32 accurate): square, column-sum by matmul
                xsq = sp.tile([C, N], f32)
                nc.vector.tensor_mul(xsq, xTaug[0:C, :], xTaug[0:C, :])
                negones = sp.tile([C, 1], f32)
                nc.vector.memset(negones, -1.0)
                sqrow = sp.tile([1, N], f32)
                for b in range(NBLK):
                    p = sps.tile([1, 512], f32, tag="sq")
                    nc.tensor.matmul(p, negones, xsq[:, b * 512:(b + 1) * 512],
                                     start=True, stop=True)
                    nc.vector.tensor_copy(sqrow[0:1, b * 512:(b + 1) * 512], p)
                nc.vector.tensor_copy(xc16[C:C + 1, :], sqrow)

                # x_dup: fp16 x duplicated into u32 lanes (rows 0:64)
                xd16 = x_dup.bitcast(f16)  # [128, 2N] view
                for half in range(2):
                    dst = bass.AP(
                        tensor=xd16.tensor,
                        offset=xd16.offset + half * xd16.ap[-1][0],
                        ap=[[xd16.ap[0][0], C], [2 * xd16.ap[-1][0], N]],
                    )
                    nc.vector.tensor_copy(dst, xq16[0:C, :])

                # v tiles: (x W_v + b1) per point, fp16, [i, T, ch]
                for t in range(nt):
                    p = sps.tile([128, 128], f32, tag="tp")
                    nc.tensor.matmul(
                        p, xTaug[:, t * 128:(t + 1) * 128], wv,
                        start=True, stop=True)
                    nc.vector.tensor_copy(v_all[:, t, :], p)

            with tc.tile_pool(name="topk", bufs=2) as tk, \
                 tc.tile_pool(name="xgp", bufs=6) as xgp, \
                 tc.tile_pool(name="mlp", bufs=5) as mp, \
                 tc.tile_pool(name="h2p", bufs=5) as h2p, \
                 tc.tile_pool(name="h3p", bufs=4) as h3p, \
                 tc.tile_pool(name="poolp", bufs=2) as pp, \
                 tc.tile_pool(name="outp", bufs=2) as op_, \
                 tc.tile_pool(name="ps_dist", bufs=2, space="PSUM") as psd, \
                 tc.tile_pool(name="ps_l1", bufs=2, space="PSUM") as ps1, \
                 tc.tile_pool(name="ps_l2", bufs=1, space="PSUM") as ps2, \
                 tc.tile_pool(name="ps_l3", bufs=1, space="PSUM") as ps3, \
                 tc.tile_pool(name="ps_ot", bufs=1, space="PSUM") as pso:

                def topk_group(g, tiles):
                    ng = len(tiles)
                    m_sb = tk.tile([128, GROUP, NCAND], f32, tag="m_sb")
                    lidx = tk.tile([128, GROUP, NCAND], u16, tag="lidx")
                    for ti, t in enumerate(tiles):
                        for b in range(NBLK):
                            pd = psd.tile([128, 512], f32, tag="dist")
                            nc.tensor.matmul(
                                pd, xq16[:, t * 128:(t + 1) * 128],
                                xc16[:, b * 512:(b + 1) * 512],
                                start=True, stop=True)
                            nc.vector.max(
                                out=m_sb[:, ti, b * 8:(b + 1) * 8], in_=pd)
                            nc.vector.max_index(
                                out=lidx[:, ti, b * 8:(b + 1) * 8],
                                in_max=m_sb[:, ti, b * 8:(b + 1) * 8],
                                in_values=pd)
                    if stage < 3:
                        return
                    # batched key build over the whole group
                    tf = tk.tile([128, GROUP, NCAND], f32, tag="tf")
                    q16t = tk.tile([128, GROUP, NCAND], u16, tag="q16")
                    qs = tk.tile([128, GROUP, NCAND], f32, tag="qs")
                    gidx = tk.tile([128, GROUP, NCAND], f32, tag="gidx")
                    keys = tk.tile([128, GROUP, NCAND], f32, tag="keys")
                    t8 = tk.tile([128, GROUP, 24], f32, tag="t8")
                    nc.vector.tensor_scalar(
                        tf[:, 0:ng, :], m_sb[:, 0:ng, :], QS, scalar2=QOFF,
                        op0=AluOp.mult, op1=AluOp.add)
                    nc.vector.tensor_scalar(
                        q16t[:, 0:ng, :], tf[:, 0:ng, :], 4095.0, scalar2=0.0,
                        op0=AluOp.min, op1=AluOp.max)
                    offs_b = bass.AP(
                        tensor=offs.tensor, offset=offs.offset,
                        ap=[offs.ap[0], [0, ng], [offs.ap[-1][0], NCAND]],
                    )
                    nc.vector.tensor_tensor(
                        out=gidx[:, 0:ng, :], in0=lidx[:, 0:ng, :],
                        in1=offs_b, op=AluOp.add)
                    nc.vector.tensor_scalar(
                        qs[:, 0:ng, :], q16t[:, 0:ng, :], 4096.0, scalar2=None,
                        op0=AluOp.mult)
                    nc.vector.tensor_tensor(
                        out=keys[:, 0:ng, :], in0=qs[:, 0:ng, :],
                        in1=gidx[:, 0:ng, :], op=AluOp.add)
                    # per-tile top-24 merge on keys
                    for ti in range(ng):
                        work = keys[:, ti, :]
                        for r in range(3):
                            nc.vector.max(out=t8[:, ti, r * 8:(r + 1) * 8],
                                          in_=work)
                            if r < 2:
                                nc.vector.match_replace(
                                    out=work,
                                    in_to_replace=t8[:, ti, r * 8:(r + 1) * 8],
                                    in_values=work, imm_value=NEG)
                    # extract global index: ilist = key - 4096*floor(key/4096).
                    # floor via RNE u16 convert of key/4096 - (0.5 - eps); the
                    # eps keeps gidx=0 (fraction exactly .5 below an int) from
                    # tying to q-1 under round-to-nearest-even.
                    qrec = tk.tile([128, GROUP, 24], u16, tag="qrec")
                    nc.vector.tensor_scalar(
                        qrec[:, 0:ng, 0:K], t8[:, 0:ng, 0:K], 1.0 / 4096.0,
                        scalar2=-0.499992, op0=AluOp.mult, op1=AluOp.add)
                    nc.vector.scalar_tensor_tensor(
                        out=ilist[:, tiles[0]:tiles[0] + ng, :],
                        in0=qrec[:, 0:ng, 0:K], scalar=-4096.0,
                        in1=t8[:, 0:ng, 0:K], op0=AluOp.mult, op1=AluOp.add)

                def fold_group(g, tiles):
                    # wrap[q, T, h*20+k] = ilist[16h+q, T, k]  (pair order h,k,q16)
                    # contiguous 20-element runs keep DMA descriptor counts low
                    ng = len(tiles)
                    ps = wrap.ap[0][0]
                    ips = ilist.ap[0][0]
                    for h in range(8):
                        dst = bass.AP(
                            tensor=wrap.tensor,
                            offset=wrap.offset + tiles[0] * 160 + h * K,
                            ap=[[ps, 16], [160, ng], [1, K]],
                        )
                        src = bass.AP(
                            tensor=ilist.tensor,
                            offset=ilist.offset + 16 * h * ips + tiles[0] * K,
                            ap=[[ips, 16], [K, ng], [1, K]],
                        )
                        eng = nc.sync if h % 2 == 0 else nc.gpsimd
                        eng.dma_start(out=dst, in_=src)
                    # replicate partitions 0:16 -> all 128 in a 3-step tree
                    for lo, n in ((16, 16), (32, 32), (64, 64)):
                        nc.sync.dma_start(
                            out=wrap[lo:lo + n, tiles[0]:tiles[0] + ng, :],
                            in_=wrap[0:n, tiles[0]:tiles[0] + ng, :])

                def gather_tile(t):
                    xg = xgp.tile([128, PAIRS], u32, tag="xg")
                    gi = nc.gpsimd.ap_gather(
                        out_ap=xg.unsqueeze(-1),
                        in_ap=x_dup.unsqueeze(-1),
                        idxs_ap=wrap[:, t, :],
                        channels=128,
                        num_elems=N,
                        d=1,
                        num_idxs=PAIRS,
                    )
                    pool_chain.append(gi.ins)
                    return xg

                def mlp_group(xgs):
                    # stage-interleaved across the group's tiles: while one
                    # tile's PSUM is being evacuated on ACT, the PE streams
                    # the other tiles' matmuls instead of stalling in FIFO
                    tiles = sorted(xgs)
                    h1s, h2s, h3s = {}, {}, {}
                    for t in tiles:
                        h1s[t] = mp.tile([C1, PAIRS], f16, tag="h1",
                                         name=f"h1_{t}")
                        h2s[t] = h2p.tile([C2, PAIRS], f16, tag="h2",
                                          name=f"h2_{t}")
                        h3s[t] = h3p.tile([C3, PAIRS], f16, tag="h3",
                                          name=f"h3_{t}")
                    for c in range(8):  # 320-pair chunks (one h each)
                        sl = slice(c * 320, (c + 1) * 320)
                        rep = bass.AP(
                            tensor=ident16.tensor,
                            offset=ident16.offset + 16 * c * ident16.ap[-1][0],
                            ap=[ident16.ap[0], [0, K], [ident16.ap[-1][0], 16]],
                        )
                        for t in tiles:
                            xgf = xgs[t].bitcast(f16)
                            es = xgf.ap[-1][0]
                            p1 = ps1.tile([C1, 320], f32, tag="l1")
                            nc.tensor.matmul(p1, v_all[:, t, :], rep,
                                             start=True, stop=False)
                            xg_ch = bass.AP(
                                tensor=xgf.tensor,
                                offset=xgf.offset + 640 * c * es,
                                ap=[[xgf.ap[0][0], 128], [2 * es, 320]],
                            )
                            nc.tensor.matmul(p1, w1t16, xg_ch,
                                             start=False, stop=True)
                            nc.scalar.activation(h1s[t][:, sl], p1, Act.Relu)
                    for cc in range(5):
                        sl = slice(cc * 512, (cc + 1) * 512)
                        for t in tiles:
                            p2 = ps2.tile([C2, 512], f32, tag="l2")
                            nc.tensor.matmul(p2, w2t16, h1s[t][:, sl],
                                             start=True, stop=True)
                            nc.scalar.activation(h2s[t][:, sl], p2, Act.Relu,
                                                 bias=b2_col)
                    for cc in range(5):
                        sl = slice(cc * 512, (cc + 1) * 512)
                        for t in tiles:
                            p3 = ps3.tile([C3, 512], f32, tag="l3")
                            nc.tensor.matmul(p3, w3t16, h2s[t][:, sl],
                                             start=True, stop=True)
                            nc.scalar.activation(h3s[t][:, sl], p3, Act.Relu,
                                                 bias=b3_col)
                    for t in tiles:
                        pool_out_tile(t, h3s[t])

                def pool_out_tile(t, h3):
                    # max over K; pair order is (h, k, q16) so use kviews
                    def kview(src, nk, k0, kn):
                        # view [64, 8h, kn(of nk), 16q] of a (h,k,q) tensor
                        s = src.ap[-1][0]
                        return bass.AP(
                            tensor=src.tensor,
                            offset=src.offset + k0 * 16 * s,
                            ap=[src.ap[0], [s * 16 * nk, 8], [s * 16, kn],
                                [s, 16]],
                        )

                    m10 = pp.tile([C3, 1280], f16, tag="m10")
                    nc.vector.tensor_tensor(
                        out=m10, in0=kview(h3, K, 0, 10),
                        in1=kview(h3, K, 10, 10), op=AluOp.max)
                    m5 = pp.tile([C3, 640], f16, tag="m5")
                    nc.vector.tensor_tensor(
                        out=m5, in0=kview(m10, 10, 0, 5),
                        in1=kview(m10, 10, 5, 5), op=AluOp.max)
                    m2 = pp.tile([C3, 256], f16, tag="m2")
                    nc.vector.tensor_tensor(
                        out=m2, in0=kview(m5, 5, 0, 2),
                        in1=kview(m5, 5, 2, 2), op=AluOp.max)
                    m1 = pp.tile([C3, 128], f16, tag="m1")
                    nc.vector.tensor_tensor(
                        out=m1, in0=kview(m2, 2, 0, 1),
                        in1=kview(m2, 2, 1, 1), op=AluOp.max)
                    pooled = pp.tile([C3, 128], f16, tag="pooled")
                    nc.vector.tensor_tensor(
                        out=pooled, in0=m1, in1=kview(m5, 5, 4, 1),
                        op=AluOp.max)
                    po = pso.tile([128, C3], f16, tag="ot")
                    nc.tensor.transpose(po, pooled, ident16[0:C3, 0:C3])
                    osb = op_.tile([128, C3], f32, tag="osb")
                    nc.vector.tensor_copy(osb, po)
                    nc.sync.dma_start(
                        out=out[t * 128:(t + 1) * 128, :], in_=osb)

                # software-pipelined: group g's fold/gather latency overlaps
                # group g-1's MLP (keeps the PE FIFO from serializing on it)
                n_groups = (nt + GROUP - 1) // GROUP
                prev = None
                for g in range(n_groups):
                    tiles = list(range(g * GROUP, min((g + 1) * GROUP, nt)))
                    topk_group(g, tiles)
                    if stage >= 4:
                        fold_group(g, tiles)
                    xgs = ({t: gather_tile(t) for t in tiles}
                           if stage >= 5 else {})
                    if prev and stage >= 6:
                        mlp_group(prev)
                    prev = xgs
                if prev and stage >= 6:
                    mlp_group(prev)

        # order GPSIMD extended-ISA ops to batch ucode-library reloads
        if os.environ.get("KM_CHAIN", "1") == "1":
            from concourse.tile_rust import add_dep_helper
            for a, b_ in zip(pool_chain, pool_chain[1:]):
                add_dep_helper(b_, a, sync=False, reason="gpsimd library batching")

    return nc


_nc_cache = None
LAST_EXEC_NS = None


def kernel(points, W1, b1, W2, b2, W3, b3):
    global _nc_cache
    if _nc_cache is None:
        _nc_cache = build_nc()
        # Bacc defers register allocation to compile(); the PJRT path
        # serializes the module as-is, so finalize before running.
        _nc_cache.finalize()
    nc = _nc_cache
    common = {
        "W1": np.ascontiguousarray(W1, dtype=np.float32),
        "b1": np.ascontiguousarray(b1, dtype=np.float32),
        "W2": np.ascontiguousarray(W2, dtype=np.float32),
        "b2": np.ascontiguousarray(b2, dtype=np.float32),
        "W3": np.ascontiguousarray(W3, dtype=np.float32),
        "b3": np.ascontiguousarray(b3, dtype=np.float32),
    }
    in_maps = [
        dict(common, points=np.ascontiguousarray(points[b], dtype=np.float32))
        for b in range(B)
    ]
    trace = os.environ.get("BASS_TRACE", "0") == "1"
    res = run_bass_kernel_spmd(nc, in_maps, list(range(B)), trace=trace)
    global LAST_EXEC_NS
    LAST_EXEC_NS = res.exec_time_ns
    return np.stack([res.results[b]["out"] for b in range(B)], axis=0)


if __name__ == "__main__":
    pts = np.random.randn(B, N, C).astype(np.float32)
    W1_ = (np.random.randn(C1, 2 * C) * 0.05).astype(np.float32)
    W2_ = (np.random.randn(C2, C1) * 0.05).astype(np.float32)
    W3_ = (np.random.randn(C3, C2) * 0.05).astype(np.float32)
    z1, z2, z3 = (np.zeros(C1, np.float32), np.zeros(C2, np.float32),
                  np.zeros(C3, np.float32))
    o = kernel(pts, W1_, z1, W2_, z2, W3_, z3)
    print(o.shape, o.dtype)


# revision 33
# speedup vs baseline: 2.3946x; 2.3946x over previous
"""DGCNN edge-conv graph-feature module on Trainium2 (Bass/Tile), v2.

Problem: for each batch (B=8): F-space KNN (k=20) over N=4096 points (C=64),
gather neighbor features, edge-MLP (128->128->64->64 with relu), max-pool
over the 20 neighbors -> (4096, 64).

Sharding: data-parallel over batch across the 8 NeuronCores (one batch each,
SPMD single NEFF).

v2 changes vs v1 (1.57 ms):
  - distance matmuls in fp16 (1 cyc/row vs 4 for fp32).
  - index compaction via quantized sort keys: key = q*4096 + j packed in
    exact-int fp32 (q = clip(round(64 d + 3072), 0, 4095)); the top-20-of-64
    merge runs on keys, so indices fall out of a cheap `mod 4096` — no
    GPSIMD local_scatter, no cumsum chain.
  - neighbor gather via gpsimd.ap_gather from a duplicated-fp16-in-u32
    SBUF table (Q7 cores move data; no per-pair DMA descriptor generation,
    which was ~800 us of SWDGE ucode in v1).
  - pair order is (k, q128) so the v-broadcast matmul uses one fixed
    0-stride identity AP and the K max-pool reduces over contiguous halves.
  - engine rebalance: PSUM evacuations on ACT (relu fused), max-pool tree +
    key build on GPSIMD, top-k scans + merge on DVE.
"""

import os
import sys

for _p in ("/opt/trn_rl_repo", "/root/.axon_site/_ro/trn_rl_repo"):
    if os.path.isdir(_p) and _p not in sys.path:
        sys.path.insert(0, _p)

import numpy as np

import concourse.bass as bass
import concourse.mybir as mybir
from concourse import bacc
from concourse.bass_utils import run_bass_kernel_spmd
from concourse.masks import make_identity
from concourse.tile import TileContext

f32 = mybir.dt.float32
f16 = mybir.dt.float16
i16 = mybir.dt.int16
u16 = mybir.dt.uint16
u32 = mybir.dt.uint32

B, N, C, K = 8, 4096, 64, 20
C1, C2, C3 = 128, 64, 64
NT = N // 128              # point tiles per core
NBLK = N // 512            # candidate blocks per tile
NCAND = NBLK * 8           # merge candidates per row
PAIRS = 128 * K            # pairs per point tile (2560)
GROUP = 4                  # tiles per key/fold/gather/mlp phase group
NEG = -1e30
QS = 64.0                  # key quantization scale
QOFF = 3072.0              # key quantization offset

AluOp = mybir.AluOpType
Act = mybir.ActivationFunctionType


def build_nc(nt=NT, stage=None):
    if stage is None:
        stage = int(os.environ.get("KM_STAGE", "9"))
    nc = bacc.Bacc(None, target_bir_lowering=False, num_swdge_queues=4)

    pts = nc.declare_dram_parameter("points", [N, C], f32, isOutput=False)
    w1 = nc.declare_dram_parameter("W1", [C1, 2 * C], f32, isOutput=False)
    b1 = nc.declare_dram_parameter("b1", [C1], f32, isOutput=False)
    w2 = nc.declare_dram_parameter("W2", [C2, C1], f32, isOutput=False)
    b2 = nc.declare_dram_parameter("b2", [C2], f32, isOutput=False)
    w3 = nc.declare_dram_parameter("W3", [C3, C2], f32, isOutput=False)
    b3 = nc.declare_dram_parameter("b3", [C3], f32, isOutput=False)
    out = nc.declare_dram_parameter("out", [N, C3], f32, isOutput=True)

    pool_chain = []  # GPSIMD extended-ISA ops, chained to batch ucode libraries

    with TileContext(nc) as tc:
        with tc.tile_pool(name="const", bufs=1) as cp:
            ident = cp.tile([128, 128], f32)
            make_identity(nc, ident)
            ident16 = cp.tile([128, 128], f16)
            nc.vector.tensor_copy(ident16, ident)

            # ---- persistent tensors
            xq16 = cp.tile([C + 1, N], f16)   # rows 0:64 = xT, row 64 = ones
            xc16 = cp.tile([C + 1, N], f16)   # rows 0:64 = 2*xT, row 64 = -|x|^2
            v_all = cp.tile([128, nt, C1], f16)
            w1t16 = cp.tile([2 * C, C1], f16)
            w2t16 = cp.tile([C1, C2], f16)
            w3t16 = cp.tile([C2, C3], f16)
            b2_col = cp.tile([C2, 1], f32)
            b3_col = cp.tile([C3, 1], f32)
            offs = cp.tile([128, NCAND], u16)  # slot -> 512*block
            ilist = cp.tile([128, nt, K], u32)
            # u_j = W1e x_j staged to DRAM for the HW-DGE indirect gather
            u_dram = nc.dram_tensor("u_dram", [N, C1], f16)

            nc.sync.dma_start(out=b2_col, in_=b2.ap().rearrange("(c a) -> c a", a=1))
            nc.sync.dma_start(out=b3_col, in_=b3.ap().rearrange("(c a) -> c a", a=1))
            nc.gpsimd.iota(offs, pattern=[[512, NBLK], [0, 8]], base=0,
                           channel_multiplier=0)
            nc.vector.memset(xq16[C:C + 1, :], 1.0)

            # ---- setup (freed after): load, transpose, weights, v
            with tc.tile_pool(name="setup", bufs=1) as sp, \
                 tc.tile_pool(name="setup_ps", bufs=2, space="PSUM") as sps:
                x_sb = sp.tile([128, nt, C], f32)
                src = bass.AP(
                    tensor=pts.ap().tensor, offset=0,
                    ap=[[C, 128], [128 * C, nt], [1, C]],
                )
                nc.sync.dma_start(out=x_sb, in_=src)

                w1_sb = sp.tile([C1, 2 * C], f32)
                nc.sync.dma_start(out=w1_sb, in_=w1[:, :])
                w2_sb = sp.tile([C2, C1], f32)
                nc.sync.dma_start(out=w2_sb, in_=w2[:, :])
                w3_sb = sp.tile([C3, C2], f32)
                nc.sync.dma_start(out=w3_sb, in_=w3[:, :])

                xTaug = sp.tile([C + 1, N], f32)
                nc.vector.memset(xTaug[C:C + 1, :], 1.0)

                # transpose W1, W2, W3 (fp16 copies)
                p = sps.tile([128, 128], f32, tag="tp")
                nc.tensor.transpose(p, w1_sb, ident)
                nc.vector.tensor_copy(w1t16, p)

                p = sps.tile([128, 128], f32, tag="tp")
                nc.tensor.transpose(p[:, 0:C2], w2_sb, ident[0:C2, 0:C2])
                nc.vector.tensor_copy(w2t16, p[0:C1, 0:C2])

                p = sps.tile([128, 128], f32, tag="tp")
                nc.tensor.transpose(p[0:C2, 0:C3], w3_sb[:, :], ident[0:C2, 0:C2])
                nc.vector.tensor_copy(w3t16, p[0:C2, 0:C3])

                # Wv_aug [65, C1]: rows 0:64 = (W1c - W1e)^T, row 64 = b1
                wv = sp.tile([C + 1, C1], f32)
                delta = sp.tile([C1, C], f32)
                nc.vector.tensor_tensor(
                    out=delta, in0=w1_sb[:, C:2 * C], in1=w1_sb[:, 0:C],
                    op=AluOp.subtract)
                p = sps.tile([128, 128], f32, tag="tp")
                nc.tensor.transpose(p[0:C, :], delta, ident)
                nc.vector.tensor_copy(wv[0:C, :], p[0:C, :])
                nc.sync.dma_start(out=wv[C:C + 1, :],
                                  in_=b1.ap().rearrange("(a c) -> a c", a=1))

                # per point-tile: transpose x -> xTaug, xq16, xc16
                for t in range(nt):
                    p = sps.tile([128, 128], f32, tag="tp")
                    nc.tensor.transpose(p[0:C, :], x_sb[:, t, :], ident)
                    sl = slice(t * 128, (t + 1) * 128)
                    nc.vector.tensor_copy(xTaug[0:C, sl], p[0:C, :])
                    nc.vector.tensor_copy(xq16[0:C, sl], p[0:C, :])
                    nc.vector.tensor_scalar_mul(xc16[0:C, sl], p[0:C, :], 2.0)

                # -|x_j|^2 row (fp32 accurate): square, column-sum by matmul
                xsq = sp.tile([C, N], f32)
                nc.vector.tensor_mul(xsq, xTaug[0:C, :], xTaug[0:C, :])
                negones = sp.tile([C, 1], f32)
                nc.vector.memset(negones, -1.0)
                sqrow = sp.tile([1, N], f32)
                for b in range(NBLK):
                    p = sps.tile([1, 512], f32, tag="sq")
                    nc.tensor.matmul(p, negones, xsq[:, b * 512:(b + 1) * 512],
                                     start=True, stop=True)
                    nc.vector.tensor_copy(sqrow[0:1, b * 512:(b + 1) * 512], p)
                nc.vector.tensor_copy(xc16[C:C + 1, :], sqrow)

                # v tiles: (x W_v + b1) per point, fp16, [i, T, ch]
                for t in range(nt):
                    p = sps.tile([128, 128], f32, tag="tp")
                    nc.tensor.matmul(
                        p, xTaug[:, t * 128:(t + 1) * 128], wv,
                        start=True, stop=True)
                    nc.vector.tensor_copy(v_all[:, t, :], p)

                # u = W1e x per point, fp16 [pt, ch], staged to DRAM
                u_st = sp.tile([128, nt, C1], f16)
                for t in range(nt):
                    p = sps.tile([128, 128], f32, tag="tp")
                    nc.tensor.matmul(
                        p, xq16[0:C, t * 128:(t + 1) * 128], w1t16[0:C, :],
                        start=True, stop=True)
                    nc.vector.tensor_copy(u_st[:, t, :], p)
                u_dst = bass.AP(
                    tensor=u_dram.ap().tensor, offset=0,
                    ap=[[C1, 128], [128 * C1, nt], [1, C1]],
                )
                u_dma = nc.sync.dma_start(out=u_dst, in_=u_st)

            with tc.tile_pool(name="topk", bufs=2) as tk, \
                 tc.tile_pool(name="xgp", bufs=6) as xgp, \
                 tc.tile_pool(name="mlp", bufs=5) as mp, \
                 tc.tile_pool(name="h2p", bufs=5) as h2p, \
                 tc.tile_pool(name="h3p", bufs=4) as h3p, \
                 tc.tile_pool(name="poolp", bufs=2) as pp, \
                 tc.tile_pool(name="outp", bufs=2) as op_, \
                 tc.tile_pool(name="ps_dist", bufs=2, space="PSUM") as psd, \
                 tc.tile_pool(name="ps_l1", bufs=2, space="PSUM") as ps1, \
                 tc.tile_pool(name="ps_l2", bufs=1, space="PSUM") as ps2, \
                 tc.tile_pool(name="ps_l3", bufs=1, space="PSUM") as ps3, \
                 tc.tile_pool(name="ps_ot", bufs=1, space="PSUM") as pso:

                def topk_group(g, tiles):
                    ng = len(tiles)
                    m_sb = tk.tile([128, GROUP, NCAND], f32, tag="m_sb")
                    lidx = tk.tile([128, GROUP, NCAND], u16, tag="lidx")
                    for ti, t in enumerate(tiles):
                        for b in range(NBLK):
                            pd = psd.tile([128, 512], f32, tag="dist")
                            nc.tensor.matmul(
                                pd, xq16[:, t * 128:(t + 1) * 128],
                                xc16[:, b * 512:(b + 1) * 512],
                                start=True, stop=True)
                            nc.vector.max(
                                out=m_sb[:, ti, b * 8:(b + 1) * 8], in_=pd)
                            nc.vector.max_index(
                                out=lidx[:, ti, b * 8:(b + 1) * 8],
                                in_max=m_sb[:, ti, b * 8:(b + 1) * 8],
                                in_values=pd)
                    if stage < 3:
                        return
                    # batched key build over the whole group
                    tf = tk.tile([128, GROUP, NCAND], f32, tag="tf")
                    q16t = tk.tile([128, GROUP, NCAND], u16, tag="q16")
                    qs = tk.tile([128, GROUP, NCAND], f32, tag="qs")
                    gidx = tk.tile([128, GROUP, NCAND], f32, tag="gidx")
                    keys = tk.tile([128, GROUP, NCAND], f32, tag="keys")
                    t8 = tk.tile([128, GROUP, 24], f32, tag="t8")
                    nc.vector.tensor_scalar(
                        tf[:, 0:ng, :], m_sb[:, 0:ng, :], QS, scalar2=QOFF,
                        op0=AluOp.mult, op1=AluOp.add)
                    nc.vector.tensor_scalar(
                        q16t[:, 0:ng, :], tf[:, 0:ng, :], 4095.0, scalar2=0.0,
                        op0=AluOp.min, op1=AluOp.max)
                    offs_b = bass.AP(
                        tensor=offs.tensor, offset=offs.offset,
                        ap=[offs.ap[0], [0, ng], [offs.ap[-1][0], NCAND]],
                    )
                    nc.vector.tensor_tensor(
                        out=gidx[:, 0:ng, :], in0=lidx[:, 0:ng, :],
                        in1=offs_b, op=AluOp.add)
                    nc.vector.tensor_scalar(
                        qs[:, 0:ng, :], q16t[:, 0:ng, :], 4096.0, scalar2=None,
                        op0=AluOp.mult)
                    nc.vector.tensor_tensor(
                        out=keys[:, 0:ng, :], in0=qs[:, 0:ng, :],
                        in1=gidx[:, 0:ng, :], op=AluOp.add)
                    # per-tile top-24 merge on keys
                    for ti in range(ng):
                        work = keys[:, ti, :]
                        for r in range(3):
                            nc.vector.max(out=t8[:, ti, r * 8:(r + 1) * 8],
                                          in_=work)
                            if r < 2:
                                nc.vector.match_replace(
                                    out=work,
                                    in_to_replace=t8[:, ti, r * 8:(r + 1) * 8],
                                    in_values=work, imm_value=NEG)
                    # extract global index: ilist = key - 4096*floor(key/4096).
                    # floor via RNE u16 convert of key/4096 - (0.5 - eps); the
                    # eps keeps gidx=0 (fraction exactly .5 below an int) from
                    # tying to q-1 under round-to-nearest-even.
                    qrec = tk.tile([128, GROUP, 24], u16, tag="qrec")
                    nc.vector.tensor_scalar(
                        qrec[:, 0:ng, 0:K], t8[:, 0:ng, 0:K], 1.0 / 4096.0,
                        scalar2=-0.499992, op0=AluOp.mult, op1=AluOp.add)
                    nc.vector.scalar_tensor_tensor(
                        out=ilist[:, tiles[0]:tiles[0] + ng, :],
                        in0=qrec[:, 0:ng, 0:K], scalar=-4096.0,
                        in1=t8[:, 0:ng, 0:K], op0=AluOp.mult, op1=AluOp.add)

                from concourse.tile_rust import add_dep_helper

                def gather_tile(t):
                    # HW indirect gather: one offset per dst partition-row, so
                    # one DMA per k slot; lands [pt, k, ch] with pair order
                    # (k, q128) after the per-k transposes in the L1 stage
                    xg = xgp.tile([128, K, C1], f16, tag="xg")
                    for k in range(K):
                        gi = nc.gpsimd.indirect_dma_start(
                            out=xg[:, k, :],
                            out_offset=None,
                            in_=u_dram[:, :],
                            in_offset=bass.IndirectOffsetOnAxis(
                                ap=ilist[:, t, k:k + 1], axis=0),
                        )
                        # DRAM dep (u_dram write) is not tracked by the tile
                        # framework; order gathers behind the staging DMA
                        add_dep_helper(gi.ins, u_dma.ins, sync=True,
                                       reason="u_dram staging before gather")
                    return xg

                def mlp_group(xgs):
                    # stage-interleaved across the group's tiles: while one
                    # tile's PSUM is being evacuated on ACT, the PE streams
                    # the other tiles' matmuls instead of stalling in FIFO
                    tiles = sorted(xgs)
                    h1s, h2s, h3s = {}, {}, {}
                    for t in tiles:
                        h1s[t] = mp.tile([C1, PAIRS], f16, tag="h1",
                                         name=f"h1_{t}")
                        h2s[t] = h2p.tile([C2, PAIRS], f16, tag="h2",
                                          name=f"h2_{t}")
                        h3s[t] = h3p.tile([C3, PAIRS], f16, tag="h3",
                                          name=f"h3_{t}")
                    for pack in range(5):  # 4 k's per 512-wide PSUM bank
                        sl = slice(pack * 512, (pack + 1) * 512)
                        for t in tiles:
                            p1 = ps1.tile([C1, 512], f32, tag="l1")
                            for r in range(4):
                                q = p1[:, r * 128:(r + 1) * 128]
                                nc.tensor.matmul(q, v_all[:, t, :], ident16,
                                                 start=True, stop=False)
                                nc.tensor.matmul(
                                    q, xgs[t][:, 4 * pack + r, :], ident16,
                                    start=False, stop=True)
                            nc.scalar.activation(h1s[t][:, sl], p1, Act.Relu)
                    for cc in range(5):
                        sl = slice(cc * 512, (cc + 1) * 512)
                        for t in tiles:
                            p2 = ps2.tile([C2, 512], f32, tag="l2")
                            nc.tensor.matmul(p2, w2t16, h1s[t][:, sl],
                                             start=True, stop=True)
                            nc.scalar.activation(h2s[t][:, sl], p2, Act.Relu,
                                                 bias=b2_col)
                    for cc in range(5):
                        sl = slice(cc * 512, (cc + 1) * 512)
                        for t in tiles:
                            p3 = ps3.tile([C3, 512], f32, tag="l3")
                            nc.tensor.matmul(p3, w3t16, h2s[t][:, sl],
                                             start=True, stop=True)
                            nc.scalar.activation(h3s[t][:, sl], p3, Act.Relu,
                                                 bias=b3_col)
                    for t in tiles:
                        pool_out_tile(t, h3s[t])

                def pool_out_tile(t, h3):
                    # max over K in contiguous (k, q128) halves
                    m10 = pp.tile([C3, 1280], f16, tag="m10")
                    nc.vector.tensor_tensor(
                        out=m10, in0=h3[:, 0:1280], in1=h3[:, 1280:2560],
                        op=AluOp.max)
                    m5 = pp.tile([C3, 640], f16, tag="m5")
                    nc.vector.tensor_tensor(
                        out=m5, in0=m10[:, 0:640], in1=m10[:, 640:1280],
                        op=AluOp.max)
                    m2 = pp.tile([C3, 256], f16, tag="m2")
                    nc.vector.tensor_tensor(
                        out=m2, in0=m5[:, 0:256], in1=m5[:, 256:512],
                        op=AluOp.max)
                    m1 = pp.tile([C3, 128], f16, tag="m1")
                    nc.vector.tensor_tensor(
                        out=m1, in0=m2[:, 0:128], in1=m2[:, 128:256],
                        op=AluOp.max)
                    pooled = pp.tile([C3, 128], f16, tag="pooled")
                    nc.vector.tensor_tensor(
                        out=pooled, in0=m1, in1=m5[:, 512:640],
                        op=AluOp.max)
                    po = pso.tile([128, C3], f16, tag="ot")
                    nc.tensor.transpose(po, pooled, ident16[0:C3, 0:C3])
                    osb = op_.tile([128, C3], f32, tag="osb")
                    nc.vector.tensor_copy(osb, po)
                    nc.sync.dma_start(
                        out=out[t * 128:(t + 1) * 128, :], in_=osb)

                # software-pipelined: group g's fold/gather latency overlaps
                # group g-1's MLP (keeps the PE FIFO from serializing on it)
                n_groups = (nt + GROUP - 1) // GROUP
                prev = None
                for g in range(n_groups):
                    tiles = list(range(g * GROUP, min((g + 1) * GROUP, nt)))
                    topk_group(g, tiles)
                    xgs = ({t: gather_tile(t) for t in tiles}
                           if stage >= 5 else {})
                    if prev and stage >= 6:
                        mlp_group(prev)
                    prev = xgs
                if prev and stage >= 6:
                    mlp_group(prev)

        # order GPSIMD extended-ISA ops to batch ucode-library reloads
        if os.environ.get("KM_CHAIN", "1") == "1":
            from concourse.tile_rust import add_dep_helper
            for a, b_ in zip(pool_chain, pool_chain[1:]):
                add_dep_helper(b_, a, sync=False, reason="gpsimd library batching")

    return nc


_nc_cache = None
LAST_EXEC_NS = None


def kernel(points, W1, b1, W2, b2, W3, b3):
    global _nc_cache
    if _nc_cache is None:
        _nc_cache = build_nc()
        # Bacc defers register allocation to compile(); the PJRT path
        # serializes the module as-is, so finalize before running.
        _nc_cache.finalize()
    nc = _nc_cache
    common = {
        "W1": np.ascontiguousarray(W1, dtype=np.float32),
        "b1": np.ascontiguousarray(b1, dtype=np.float32),
        "W2": np.ascontiguousarray(W2, dtype=np.float32),
        "b2": np.ascontiguousarray(b2, dtype=np.float32),
        "W3": np.ascontiguousarray(W3, dtype=np.float32),
        "b3": np.ascontiguousarray(b3, dtype=np.float32),
    }
    in_maps = [
        dict(common, points=np.ascontiguousarray(points[b], dtype=np.float32))
        for b in range(B)
    ]
    trace = os.environ.get("BASS_TRACE", "0") == "1"
    res = run_bass_kernel_spmd(nc, in_maps, list(range(B)), trace=trace)
    global LAST_EXEC_NS
    LAST_EXEC_NS = res.exec_time_ns
    return np.stack([res.results[b]["out"] for b in range(B)], axis=0)


if __name__ == "__main__":
    pts = np.random.randn(B, N, C).astype(np.float32)
    W1_ = (np.random.randn(C1, 2 * C) * 0.05).astype(np.float32)
    W2_ = (np.random.randn(C2, C1) * 0.05).astype(np.float32)
    W3_ = (np.random.randn(C3, C2) * 0.05).astype(np.float32)
    z1, z2, z3 = (np.zeros(C1, np.float32), np.zeros(C2, np.float32),
                  np.zeros(C3, np.float32))
    o = kernel(pts, W1_, z1, W2_, z2, W3_, z3)
    print(o.shape, o.dtype)
